# revision 48
# baseline (speedup 1.0000x reference)
"""Trainium2 Bass kernel for nn_CLLayer (SimCLR-style contrastive loss).

Math (reference, tau=0.5):
    h1 = elu(z1 @ W1.T + b1) @ W2.T + b2 ; h2 likewise
    n1, n2 = row-normalized h1, h2
    l1_i = log(sum_j exp(2*n1_i.n1_j) + sum_j exp(2*n1_i.n2_j) - e^2) - 2*n1_i.n2_i
    l2_i = log(sum_j exp(2*n2_i.n2_j) + sum_j exp(2*n2_j.n1_i... ) - e^2) - 2*...
    out = 0.5*(l1+l2)

Sharding: row-parallel over N=8192 (1024 rows/core, 8 cores).
Each core: projects its row block, normalizes, AllGathers normalized
embeddings, computes its row-strip of the three distinct similarity
products (S12, S22, S11), exp+row-sums on the fly, column-sums of
exp(2*S12) via a ReduceScatter (between2 = between.T so l2's "between"
row sums are column sums of S12's exp).  Only 3 of 4 N^2*D products run.

All matmuls are fp8e4 with perf_mode=DoubleRow (2x PE throughput;
each MM consumes a [P, 2, x] K-slab pair, K=256).  fp8 subnormals are
avoided by x16 pre-scales: weights are scaled x16 on the host (undone
via the activation `scale`), normalized embeddings x16 on device
(undone in the exp scale 2/256 and the positive-pair term -2/256).
Each AllGather is split into two column halves so pass A can start on
the first half while the second is still in flight.

Host-side prep: transposes z blocks / weights to K-major (PE wants K on
partitions), casts matmul operands to fp8e4 (ml_dtypes.float8_e4m3
matches TRN FP8_EXP4 bit-exactly below 240), and folds the ELU "-1"
into an adjusted fc2 bias (b2' = b2 - fc2_w.sum(1)) so ELU is computed
as relu(x) + min(exp(x),1) without the subtract (device ELU' = elu+1).
"""

import math
import os
from functools import lru_cache

import ml_dtypes
import numpy as np

import concourse.bacc as bacc
import concourse.bass as bass
import concourse.mybir as mybir
import concourse.tile as tile
from concourse.bass_utils import run_bass_kernel_spmd

N, D = 8192, 1024
NCORES = 8
BLK = N // NCORES  # 1024
P = 128
KO = D // P  # 8 k-tiles
NT = BLK // P  # 8 i-tiles per core
JP = NCORES  # 8 j-chunks of 1024 (= core blocks)
E2 = float(np.exp(2.0))  # exp(1/tau), tau=0.5
BF = mybir.dt.bfloat16
F8 = mybir.dt.float8e4
F32 = mybir.dt.float32
NS = 16.0  # fp8 pre-scale on normalized embeddings
WS = 16.0  # fp8 pre-scale on weights (host side)
DR = mybir.MatmulPerfMode.DoubleRow
AF = mybir.ActivationFunctionType
ALU = mybir.AluOpType


def _build():
    nc = bacc.Bacc("TRN2", target_bir_lowering=False, debug=False, num_devices=NCORES)

    z1t = nc.dram_tensor("z1t", [D, BLK], F8, kind="ExternalInput")
    z2t = nc.dram_tensor("z2t", [D, BLK], F8, kind="ExternalInput")
    w1t = nc.dram_tensor("w1t", [D, D], F8, kind="ExternalInput")
    w2t = nc.dram_tensor("w2t", [D, D], F8, kind="ExternalInput")
    b1 = nc.dram_tensor("b1", [D], F32, kind="ExternalInput")
    b2p = nc.dram_tensor("b2p", [D], F32, kind="ExternalInput")
    out = nc.dram_tensor("out", [BLK], F32, kind="ExternalOutput")

    kp = lambda ap: ap.rearrange("(ko ki) x -> ki ko x", ki=P)  # K-major -> [128, KO, x]
    pt = lambda ap: ap.rearrange("(t p) -> p t", p=P)  # [1024] -> [128, 8]

    with tile.TileContext(nc) as tc:
        with (
            tc.tile_pool(name="consts", bufs=1) as consts,
            tc.tile_pool(name="mats", bufs=1) as mats,
            tc.tile_pool(name="strip", bufs=1) as strip,
            tc.tile_pool(name="scratch", bufs=2) as scratch,
            tc.tile_pool(name="rhs", bufs=4) as rhsp,
            tc.tile_pool(name="expp", bufs=2) as expp,
            tc.tile_pool(name="small", bufs=1) as small,
            tc.tile_pool(name="psA", bufs=3, space="PSUM") as psA,
            tc.tile_pool(name="psB", bufs=2, space="PSUM") as psB,
            tc.tile_pool(name="dram", bufs=1, space="DRAM") as dram,
        ):
            # ---------------- constants / inputs (proj1's needs first) ----------------
            w1_sb = consts.tile([P, KO, D], F8)
            w2_sb = consts.tile([P, KO, D], F8)
            b1_sb = consts.tile([P, KO], F32)
            b2_sb = consts.tile([P, KO], F32)
            z_sb = mats.tile([P, KO, BLK], F8, tag="zt")
            # halved transfers so proj1's first matmuls start sooner
            nc.sync.dma_start(w1_sb[:, :, 0:512], kp(w1t[:])[:, :, 0:512])
            nc.sync.dma_start(z_sb[:, :, 0:512], kp(z1t[:])[:, :, 0:512])
            nc.sync.dma_start(w1_sb[:, :, 512:1024], kp(w1t[:])[:, :, 512:1024])
            nc.sync.dma_start(z_sb[:, :, 512:1024], kp(z1t[:])[:, :, 512:1024])
            nc.sync.dma_start(b1_sb[:], pt(b1[:]))
            nc.sync.dma_start(w2_sb[:], kp(w2t[:]))
            nc.sync.dma_start(b2_sb[:], pt(b2p[:]))
            # own tag: staging must not pin rhs-pool slots (WAR on the pool
            # rotation would stall pass A's prefetch until proj2-ch1 retires)
            z2a = rhsp.tile([P, KO, 512], F8, tag="zstage", name="z2a", bufs=2)
            z2b = rhsp.tile([P, KO, 512], F8, tag="zstage", name="z2b", bufs=2)
            nc.sync.dma_start(z2a[:], kp(z2t[:, 0:512]))
            nc.sync.dma_start(z2b[:], kp(z2t[:, 512:1024]))
            ones_bf = consts.tile([P, 1], BF)
            nc.vector.memset(ones_bf[:], 1.0)
            lnns = consts.tile([1, 1], F32)
            nc.vector.memset(lnns[:], float(math.log(NS)))

            n1_sb = mats.tile([P, KO, BLK], BF, tag="n1")
            n2_sb = mats.tile([P, KO, BLK], BF, tag="n2")

            ag_in = {}
            ag_out = {}
            for t in (1, 2):
                for h in (0, 1):
                    ag_in[t, h] = dram.tile([D, 512], F8, name=f"ag{t}{h}_in")
                    ag_out[t, h] = dram.tile(
                        [NCORES, D, 512], F8, addr_space="Shared", name=f"ag{t}{h}_out"
                    )
            rs_in = dram.tile([N], F32)
            rs_out = dram.tile([BLK], F32)
            rn_dram = dram.tile([2, BLK], BF)
            p_dram = dram.tile([BLK], F32)

            rg = [list(range(NCORES))]

            # ------------ projection + normalize (into n_sb + n_f8), per tensor ------------
            # Column-half-outer: each 512-column half runs L1 -> L2 -> sumsq ->
            # rn -> fp8 cast -> its AllGather trigger before the other half
            # starts, so AG h=0 is in flight ~half a projection early and
            # collective-duration variance hides under the remaining compute.
            def proj_l1(z_at, elu_sb, ch):
                # layer 1: a1T[o, i] = W1T.T @ zT (K=d);
                # elu+1 = relu(y) + min(exp(y), 1), y = ps/WS + b1
                sl = bass.ds(ch * 512, 512)
                for ot in range(KO):
                    ps = psA.tile([P, 512], F32, tag="ps_big", name="ps_l1")
                    for kt in range(0, KO, 2):
                        nc.tensor.matmul(
                            ps[:],
                            w1_sb[:, kt : kt + 2, bass.ts(ot, P)],
                            z_at(kt, ch),
                            start=(kt == 0),
                            stop=(kt == KO - 2),
                            perf_mode=DR,
                        )
                    bcol = b1_sb[:, ot : ot + 1]
                    e_t = scratch.tile([P, 512], F32, tag="e_t")
                    r_t = scratch.tile([P, 512], F32, tag="r_t")
                    nc.scalar.activation(e_t[:], ps[:], AF.Exp, bias=bcol, scale=1.0 / WS)
                    nc.scalar.activation(r_t[:], ps[:], AF.Relu, bias=bcol, scale=1.0 / WS)
                    nc.vector.tensor_scalar(e_t[:], e_t[:], 1.0, None, ALU.min)
                    nc.vector.tensor_tensor(elu_sb[:, ot, sl], e_t[:], r_t[:], ALU.add)

            def proj_l2_tail(elu_sb, n_sb, n_f8, rn_slot, t, ch):
                sl = bass.ds(ch * 512, 512)
                # layer 2 -> n_sb (holds hT until scaled)
                ssps = psB.tile([1, 512], F32, name=f"ssps{t}{ch}", tag="ps_small")
                for ot in range(KO):
                    ps = psA.tile([P, 512], F32, tag="ps_big", name="ps_l2")
                    for kt in range(0, KO, 2):
                        nc.tensor.matmul(
                            ps[:],
                            w2_sb[:, kt : kt + 2, bass.ts(ot, P)],
                            elu_sb[:, kt : kt + 2, sl],
                            start=(kt == 0),
                            stop=(kt == KO - 2),
                            perf_mode=DR,
                        )
                    nc.vector.tensor_scalar(
                        n_sb[:, ot, sl], ps[:], 1.0 / WS, b2_sb[:, ot : ot + 1],
                        ALU.mult, ALU.add,
                    )
                    # sumsq over d (partitions) via ones-matmul on Square(h)
                    sq = scratch.tile([P, 512], BF, tag="sq")
                    nc.scalar.activation(sq[:], n_sb[:, ot, sl], AF.Square)
                    nc.tensor.matmul(
                        ssps[:], ones_bf[:], sq[:],
                        start=(ot == 0), stop=(ot == KO - 1),
                    )
                # rn = NS/||h||: rsqrt = NS*Exp(-0.5*Ln(s)) on the ACT
                # tables (DVE reciprocal is single-lane slow; the Ln/Exp
                # tables already bound the kernel's overall accuracy)
                l_c = small.tile([1, 512], F32, tag="l_c", name=f"l_c{t}{ch}", bufs=2)
                rn_c = small.tile([1, 512], BF, tag="rn_c", name=f"rn_c{t}{ch}", bufs=2)
                nc.scalar.activation(l_c[:], ssps[:], AF.Ln)
                nc.scalar.activation(rn_c[:], l_c[:], AF.Exp, scale=-0.5, bias=lnns[:])
                nc.scalar.dma_start(rn_dram[rn_slot : rn_slot + 1, sl], rn_c[:])
                rn_bc = scratch.tile([P, 512], BF, tag="rnbc", bufs=2, name=f"rn_bc{t}{ch}")
                nc.scalar.dma_start(
                    rn_bc[:],
                    rn_dram[rn_slot : rn_slot + 1, sl].to_broadcast((P, 512)),
                )
                for kt in range(KO):
                    nc.vector.tensor_tensor(
                        n_f8[:, kt, sl], n_sb[:, kt, sl], rn_bc[:], ALU.mult
                    )
                nc.scalar.dma_start(kp(ag_in[t, ch][:]), n_f8[:, :, sl])
                nc.gpsimd.collective_compute(
                    "AllGather", ALU.bypass, replica_groups=rg,
                    ins=[ag_in[t, ch][:].opt()], outs=[ag_out[t, ch][:].opt()],
                )

            elu1 = mats.tile([P, KO, BLK], F8, tag="elu")
            # own slots: n_f8 ch-0 writes must not WAR-wait on the elu/z slots'
            # ch-1 readers, or the early AllGather trigger serializes away
            n1_f8 = mats.tile([P, KO, BLK], F8, tag="n1f8", name="n1_f8")
            elu2 = mats.tile([P, KO, BLK], F8, tag="elu2", name="elu2")
            n2_f8 = mats.tile([P, KO, BLK], F8, tag="n2f8", name="n2_f8")
            z1_at = lambda kt, ch: z_sb[:, kt : kt + 2, bass.ds(ch * 512, 512)]
            z2_at = lambda kt, ch: (z2a if ch == 0 else z2b)[:, kt : kt + 2, :]
            # layer-interleaved: while one phase's activation/normalize tail
            # drains, the next phase's matmuls keep the PE (and its HAM clock)
            # busy; AllGathers trigger at the end of each L2 tail in order
            # AG1a, AG2a, AG1b, AG2b
            for ch in (0, 1):
                proj_l1(z1_at, elu1, ch)
                proj_l1(z2_at, elu2, ch)
                proj_l2_tail(elu1, n1_sb, n1_f8, 0, 1, ch)
                proj_l2_tail(elu2, n2_sb, n2_f8, 1, 2, ch)


            # rowsum partials, one column per (half, jp-pair); S11 and S12
            # share one tile so a single reduce yields r11+r12
            r1x = strip.tile([P, NT, 2 * JP], F32)
            r22p = strip.tile([P, NT, JP], F32)
            cs = strip.tile([P, N], F32)  # exp(2*S12) partial column sums

            def rhs_one(t, h, jp, tag_n):
                r = rhsp.tile([P, KO, 512], F8, tag="rhs", name=f"rhs_{tag_n}")
                nc.sync.dma_start(r[:], kp(ag_out[t, h][jp]))
                return r

            def sim_iter(lhs, tt, rt0, rt1, accum, cs_sl=None):
                # fp8 DoubleRow: each matmul consumes a [P, 2, x] K-slab pair
                # (K=256); dots carry the NS^2 scale, undone in the exp scale.
                ps = psA.tile([P, 1024], F32, tag="ps_big", name="ps_sim")
                for kt in range(0, KO, 2):
                    for ch, rt in ((0, rt0), (1, rt1)):
                        nc.tensor.matmul(
                            ps[:, bass.ts(ch, 512)],
                            lhs[:, kt : kt + 2, bass.ts(tt, P)],
                            rt[:, kt : kt + 2, :],
                            start=(kt == 0),
                            stop=(kt == KO - 2),
                            perf_mode=DR,
                        )
                ex = expp.tile([P, 1024], F32, tag="ex")
                nc.scalar.activation(
                    ex[:], ps[:], AF.Exp, scale=2.0 / (NS * NS), accum_out=accum
                )
                if cs_sl is not None:
                    for ch in range(2):
                        nc.vector.tensor_tensor(
                            cs_sl[ch], cs_sl[ch], ex[:, bass.ts(ch, 512)], ALU.add
                        )

            def colsum_flush(jp, h):
                # cs chunk (jp, h) complete -> bf16 stage, reduce over partitions,
                # ship to the ReduceScatter input at its global-j offset.
                g = jp * 1024 + h * 512
                csb = scratch.tile([P, 512], BF, tag="csb", bufs=2, name=f"csb{jp}_{h}")
                nc.vector.tensor_copy(csb[:], cs[:, bass.ds(g, 512)])
                cp = psB.tile([1, 512], F32, tag="ps_small", name=f"cp{jp}_{h}")
                nc.tensor.matmul(cp[:], ones_bf[:], csb[:], start=True, stop=True)
                cst = scratch.tile([1, 512], F32, tag="cst", bufs=2, name=f"cst{jp}_{h}")
                nc.vector.tensor_copy(cst[:], cp[:])
                nc.gpsimd.dma_start(rs_in[g : g + 512], cst[:])

            def sim_pass(lhs, t, racc, is_s12, col_base=0):
                # phase h=0 runs entirely on the first AllGather half so it can
                # start before the second half lands; pair chunks share one exp.
                col = col_base
                for h in (0, 1):
                    for pj in range(0, JP, 2):
                        rt0 = rhs_one(t, h, pj, f"{t}{h}{pj}")
                        rt1 = rhs_one(t, h, pj + 1, f"{t}{h}{pj + 1}")
                        for tt in range(NT):
                            cs_sl = None
                            if is_s12:
                                cs_sl = (
                                    cs[:, bass.ds(pj * 1024 + h * 512, 512)],
                                    cs[:, bass.ds((pj + 1) * 1024 + h * 512, 512)],
                                )
                            sim_iter(lhs, tt, rt0, rt1, racc[:, tt, col : col + 1], cs_sl)
                        if is_s12:
                            colsum_flush(pj, h)
                            colsum_flush(pj + 1, h)
                        col += 1

            # ---------------- p_i = n1_i . n2_i (local diag of S12, x NS^2) ----------------
            pps = [psB.tile([1, 512], F32, name=f"pps{_c}", tag="ps_small") for _c in range(2)]
            for kt in range(KO):
                q = scratch.tile([P, BLK], BF, tag="sq")
                nc.vector.tensor_tensor(q[:], n1_f8[:, kt, :], n2_f8[:, kt, :], ALU.mult)
                for ch in range(2):
                    nc.tensor.matmul(
                        pps[ch][:],
                        ones_bf[:],
                        q[:, bass.ts(ch, 512)],
                        start=(kt == 0),
                        stop=(kt == KO - 1),
                    )
            for ch in range(2):
                p_c = small.tile([1, 512], F32, tag="p_c", name=f"p_c{ch}", bufs=2)
                nc.vector.tensor_copy(p_c[:], pps[ch][:])
                nc.gpsimd.dma_start(p_dram[ch * 512 : (ch + 1) * 512], p_c[:])
            # ---- pass B1: S12 (lhs n1, rhs gathered n2) + incremental colsums ----
            nc.vector.memset(cs[:], 0.0)
            sim_pass(n1_f8, 2, r1x, True, col_base=JP)
            nc.gpsimd.collective_compute(
                "ReduceScatter", ALU.add, replica_groups=rg,
                ins=[rs_in[:].opt()], outs=[rs_out[:].opt()],
            )
            # ---- pass A: S11 (lhs n1, rhs gathered n1) ----
            sim_pass(n1_f8, 1, r1x, False)


            # ---- pass B2: S22 (lhs n2, rhs gathered n2); RS overlaps this ----
            sim_pass(n2_f8, 2, r22p, False)

            # ---------------- final loss:  0.5*ln(d1*d2) - 2*p/NS^2 ----------------
            c12 = small.tile([P, NT], F32, tag="c12")
            nc.sync.dma_start(c12[:], pt(rs_out[:]))
            p2 = small.tile([P, NT], F32, tag="p2")
            nc.sync.dma_start(p2[:], pt(p_dram[:]))
            pm = small.tile([P, NT], F32, tag="pm")
            nc.vector.tensor_scalar(pm[:], p2[:], -2.0 / (NS * NS), None, ALU.mult)

            d1 = small.tile([P, NT], F32, tag="d1")
            d2 = small.tile([P, NT], F32, tag="d2")
            nc.vector.reduce_sum(d1[:], r1x[:], axis=mybir.AxisListType.X)
            nc.vector.tensor_scalar(d1[:], d1[:], -E2, None, ALU.add)
            nc.vector.reduce_sum(d2[:], r22p[:], axis=mybir.AxisListType.X)
            nc.vector.tensor_tensor(d2[:], d2[:], c12[:], ALU.add)
            nc.vector.tensor_scalar(d2[:], d2[:], -E2, None, ALU.add)
            nc.vector.tensor_tensor(d1[:], d1[:], d2[:], ALU.mult)
            lns = small.tile([P, NT], F32, tag="lns")
            nc.scalar.activation(lns[:], d1[:], AF.Ln)
            loss = small.tile([P, NT], F32, tag="loss")
            nc.vector.tensor_scalar(loss[:], lns[:], 0.5, None, ALU.mult)
            nc.vector.tensor_tensor(loss[:], loss[:], pm[:], ALU.add)
            nc.sync.dma_start(pt(out[:]), loss[:])

    nc.finalize()
    return nc


@lru_cache(maxsize=1)
def _built():
    return _build()


def _prep_inputs(z1, z2, fc1_w, fc1_b, fc2_w, fc2_b):
    f8 = ml_dtypes.float8_e4m3  # TRN FP8_EXP4-compatible below +-240
    w1t = np.ascontiguousarray(np.asarray(fc1_w, np.float32).T * WS).astype(f8)
    w2t = np.ascontiguousarray(np.asarray(fc2_w, np.float32).T * WS).astype(f8)
    b1 = np.asarray(fc1_b, np.float32)
    b2p = (np.asarray(fc2_b, np.float32) - np.asarray(fc2_w, np.float32).sum(axis=1)).astype(
        np.float32
    )
    in_maps = []
    for c in range(NCORES):
        sl = slice(c * BLK, (c + 1) * BLK)
        in_maps.append(
            {
                "z1t": np.ascontiguousarray(np.asarray(z1[sl], np.float32).T).astype(f8),
                "z2t": np.ascontiguousarray(np.asarray(z2[sl], np.float32).T).astype(f8),
                "w1t": w1t,
                "w2t": w2t,
                "b1": b1,
                "b2p": b2p,
            }
        )
    return in_maps


def _install_ntff_shim():
    """Register the axon NTFF profile hook (antenv.axon_hooks is absent in
    this image; rebuild it from trn_agent_boot's ctypes recipe)."""
    import sys
    import types

    if "antenv.axon_hooks" in sys.modules:
        return True
    try:
        import antenv
        from trn_agent_boot.trn_boot import _ntff_profile_via_ctypes

        hook = _ntff_profile_via_ctypes("/opt/axon/libaxon_pjrt.so")
        if hook is None:
            return False
        m = types.ModuleType("antenv.axon_hooks")
        m._hook = hook
        m.get_axon_ntff_profile_hook = lambda: m._hook
        m.set_axon_ntff_profile_hook = lambda h: setattr(m, "_hook", h)
        sys.modules["antenv.axon_hooks"] = m
        antenv.axon_hooks = m
        # artifact upload needs egress; neuter it for local profiling
        import concourse.bass_utils as _bu

        _bu.upload_artifacts = lambda tmpdir: f"file://{tmpdir}"
        return True
    except Exception as e:
        print(f"ntff shim unavailable: {e!r}")
        return False


def _run(in_maps, trace=False):
    nc = _built()
    if trace and not _install_ntff_shim():
        trace = False
    last = None
    for attempt in range(3):
        try:
            res = run_bass_kernel_spmd(nc, in_maps, list(range(NCORES)), trace=trace)
            if all(np.isfinite(res.results[c]["out"]).all() for c in range(NCORES)):
                return res
            print("nonfinite output, retrying")
        except Exception as e:  # device occasionally wedged from a prior process
            last = e
            if "UNRECOVERABLE" not in str(e) and "UNAVAILABLE" not in str(e):
                raise
            print(f"device error (attempt {attempt}): retrying")
    if last is not None:
        raise last
    return res


def kernel(z1, z2, fc1_w, fc1_b, fc2_w, fc2_b):
    in_maps = _prep_inputs(z1, z2, fc1_w, fc1_b, fc2_w, fc2_b)
    res = _run(in_maps, trace=os.environ.get("KERNEL_TRACE", "") == "1")
    if res.exec_time_ns is not None:
        print(f"HW exec time: {res.exec_time_ns} ns")
    out = np.concatenate([res.results[c]["out"] for c in range(NCORES)])
    return out.astype(np.float32)


# revision 49
# speedup vs baseline: 1.0303x; 1.0303x over previous
"""Trainium2 Bass kernel for nn_CLLayer (SimCLR-style contrastive loss).

Math (reference, tau=0.5):
    h1 = elu(z1 @ W1.T + b1) @ W2.T + b2 ; h2 likewise
    n1, n2 = row-normalized h1, h2
    l1_i = log(sum_j exp(2*n1_i.n1_j) + sum_j exp(2*n1_i.n2_j) - e^2) - 2*n1_i.n2_i
    l2_i = log(sum_j exp(2*n2_i.n2_j) + sum_j exp(2*n2_j.n1_i... ) - e^2) - 2*...
    out = 0.5*(l1+l2)

Sharding: row-parallel over N=8192 (1024 rows/core, 8 cores).
Each core: projects its row block, normalizes, AllGathers normalized
embeddings, computes its row-strip of the three distinct similarity
products (S12, S22, S11), exp+row-sums on the fly, column-sums of
exp(2*S12) via a ReduceScatter (between2 = between.T so l2's "between"
row sums are column sums of S12's exp).  Only 3 of 4 N^2*D products run.

All matmuls are fp8e4 with perf_mode=DoubleRow (2x PE throughput;
each MM consumes a [P, 2, x] K-slab pair, K=256).  fp8 subnormals are
avoided by x16 pre-scales: weights are scaled x16 on the host (undone
via the activation `scale`), normalized embeddings x16 on device
(undone in the exp scale 2/256 and the positive-pair term -2/256).
Each AllGather is split into two column halves so pass A can start on
the first half while the second is still in flight.

Host-side prep: transposes z blocks / weights to K-major (PE wants K on
partitions), casts matmul operands to fp8e4 (ml_dtypes.float8_e4m3
matches TRN FP8_EXP4 bit-exactly below 240), and folds the ELU "-1"
into an adjusted fc2 bias (b2' = b2 - fc2_w.sum(1)) so ELU is computed
as relu(x) + min(exp(x),1) without the subtract (device ELU' = elu+1).
"""

import math
import os
from functools import lru_cache

import ml_dtypes
import numpy as np

import concourse.bacc as bacc
import concourse.bass as bass
import concourse.mybir as mybir
import concourse.tile as tile
from concourse.bass_utils import run_bass_kernel_spmd

N, D = 8192, 1024
NCORES = 8
BLK = N // NCORES  # 1024
P = 128
KO = D // P  # 8 k-tiles
NT = BLK // P  # 8 i-tiles per core
JP = NCORES  # 8 j-chunks of 1024 (= core blocks)
E2 = float(np.exp(2.0))  # exp(1/tau), tau=0.5
BF = mybir.dt.bfloat16
F8 = mybir.dt.float8e4
F32 = mybir.dt.float32
NS = 16.0  # fp8 pre-scale on normalized embeddings
WS = 16.0  # fp8 pre-scale on weights (host side)
DR = mybir.MatmulPerfMode.DoubleRow
AF = mybir.ActivationFunctionType
ALU = mybir.AluOpType


def _build():
    nc = bacc.Bacc("TRN2", target_bir_lowering=False, debug=False, num_devices=NCORES)

    z1t = nc.dram_tensor("z1t", [D, BLK], F8, kind="ExternalInput")
    z2t = nc.dram_tensor("z2t", [D, BLK], F8, kind="ExternalInput")
    w1t = nc.dram_tensor("w1t", [D, D], F8, kind="ExternalInput")
    w2t = nc.dram_tensor("w2t", [D, D], F8, kind="ExternalInput")
    b1 = nc.dram_tensor("b1", [D], F32, kind="ExternalInput")
    b2p = nc.dram_tensor("b2p", [D], F32, kind="ExternalInput")
    out = nc.dram_tensor("out", [BLK], F32, kind="ExternalOutput")

    kp = lambda ap: ap.rearrange("(ko ki) x -> ki ko x", ki=P)  # K-major -> [128, KO, x]
    pt = lambda ap: ap.rearrange("(t p) -> p t", p=P)  # [1024] -> [128, 8]

    with tile.TileContext(nc) as tc:
        with (
            tc.tile_pool(name="consts", bufs=1) as consts,
            tc.tile_pool(name="mats", bufs=1) as mats,
            tc.tile_pool(name="strip", bufs=1) as strip,
            tc.tile_pool(name="scratch", bufs=2) as scratch,
            tc.tile_pool(name="rhs", bufs=4) as rhsp,
            tc.tile_pool(name="expp", bufs=2) as expp,
            tc.tile_pool(name="small", bufs=1) as small,
            tc.tile_pool(name="psA", bufs=3, space="PSUM") as psA,
            tc.tile_pool(name="psB", bufs=2, space="PSUM") as psB,
            tc.tile_pool(name="dram", bufs=1, space="DRAM") as dram,
        ):
            # ---------------- constants / inputs (proj1's needs first) ----------------
            w1_sb = consts.tile([P, KO, D], F8)
            w2_sb = consts.tile([P, KO, D], F8)
            b1_sb = consts.tile([P, KO], F32)
            b2_sb = consts.tile([P, KO], F32)
            z_sb = mats.tile([P, KO, BLK], F8, tag="zt")
            # halved transfers so proj1's first matmuls start sooner
            nc.sync.dma_start(w1_sb[:, :, 0:512], kp(w1t[:])[:, :, 0:512])
            nc.sync.dma_start(z_sb[:, :, 0:512], kp(z1t[:])[:, :, 0:512])
            nc.sync.dma_start(w1_sb[:, :, 512:1024], kp(w1t[:])[:, :, 512:1024])
            nc.sync.dma_start(z_sb[:, :, 512:1024], kp(z1t[:])[:, :, 512:1024])
            nc.sync.dma_start(b1_sb[:], pt(b1[:]))
            nc.sync.dma_start(w2_sb[:], kp(w2t[:]))
            nc.sync.dma_start(b2_sb[:], pt(b2p[:]))
            # own tag: staging must not pin rhs-pool slots (WAR on the pool
            # rotation would stall pass A's prefetch until proj2-ch1 retires)
            z2a = rhsp.tile([P, KO, 512], F8, tag="zstage", name="z2a", bufs=2)
            z2b = rhsp.tile([P, KO, 512], F8, tag="zstage", name="z2b", bufs=2)
            nc.sync.dma_start(z2a[:], kp(z2t[:, 0:512]))
            nc.sync.dma_start(z2b[:], kp(z2t[:, 512:1024]))
            ones_bf = consts.tile([P, 1], BF)
            nc.vector.memset(ones_bf[:], 1.0)
            lnns = consts.tile([1, 1], F32)
            nc.vector.memset(lnns[:], float(math.log(NS)))

            n1_sb = mats.tile([P, KO, BLK], BF, tag="n1")
            n2_sb = mats.tile([P, KO, BLK], BF, tag="n2")

            ag_in = {}
            ag_out = {}
            for t in (1, 2):
                for h in (0, 1):
                    ag_in[t, h] = dram.tile([D, 512], F8, name=f"ag{t}{h}_in")
                    ag_out[t, h] = dram.tile(
                        [NCORES, D, 512], F8, addr_space="Shared", name=f"ag{t}{h}_out"
                    )
            rs_in = dram.tile([N], F32)
            rs_out = dram.tile([BLK], F32)
            rn_dram = dram.tile([2, BLK], BF)
            p_dram = dram.tile([BLK], F32)

            rg = [list(range(NCORES))]

            # ------------ projection + normalize (into n_sb + n_f8), per tensor ------------
            # Column-half-outer: each 512-column half runs L1 -> L2 -> sumsq ->
            # rn -> fp8 cast -> its AllGather trigger before the other half
            # starts, so AG h=0 is in flight ~half a projection early and
            # collective-duration variance hides under the remaining compute.
            def proj_l1(z_at, elu_sb, ch):
                # layer 1: a1T[o, i] = W1T.T @ zT (K=d);
                # elu+1 = relu(y) + min(exp(y), 1), y = ps/WS + b1
                sl = bass.ds(ch * 512, 512)
                for ot in range(KO):
                    ps = psA.tile([P, 512], F32, tag="ps_big", name="ps_l1")
                    for kt in range(0, KO, 2):
                        nc.tensor.matmul(
                            ps[:],
                            w1_sb[:, kt : kt + 2, bass.ts(ot, P)],
                            z_at(kt, ch),
                            start=(kt == 0),
                            stop=(kt == KO - 2),
                            perf_mode=DR,
                        )
                    bcol = b1_sb[:, ot : ot + 1]
                    e_t = scratch.tile([P, 512], F32, tag="e_t")
                    r_t = scratch.tile([P, 512], F32, tag="r_t")
                    nc.scalar.activation(e_t[:], ps[:], AF.Exp, bias=bcol, scale=1.0 / WS)
                    nc.scalar.activation(r_t[:], ps[:], AF.Relu, bias=bcol, scale=1.0 / WS)
                    nc.vector.tensor_scalar(e_t[:], e_t[:], 1.0, None, ALU.min)
                    nc.vector.tensor_tensor(elu_sb[:, ot, sl], e_t[:], r_t[:], ALU.add)

            def proj_l2_tail(elu_sb, n_sb, n_f8, rn_slot, t, ch):
                sl = bass.ds(ch * 512, 512)
                # layer 2 -> n_sb (holds hT until scaled)
                ssps = psB.tile([1, 512], F32, name=f"ssps{t}{ch}", tag="ps_small")
                for ot in range(KO):
                    ps = psA.tile([P, 512], F32, tag="ps_big", name="ps_l2")
                    for kt in range(0, KO, 2):
                        nc.tensor.matmul(
                            ps[:],
                            w2_sb[:, kt : kt + 2, bass.ts(ot, P)],
                            elu_sb[:, kt : kt + 2, sl],
                            start=(kt == 0),
                            stop=(kt == KO - 2),
                            perf_mode=DR,
                        )
                    nc.vector.tensor_scalar(
                        n_sb[:, ot, sl], ps[:], 1.0 / WS, b2_sb[:, ot : ot + 1],
                        ALU.mult, ALU.add,
                    )
                    # sumsq over d (partitions) via ones-matmul on Square(h)
                    sq = scratch.tile([P, 512], BF, tag="sq")
                    nc.scalar.activation(sq[:], n_sb[:, ot, sl], AF.Square)
                    nc.tensor.matmul(
                        ssps[:], ones_bf[:], sq[:],
                        start=(ot == 0), stop=(ot == KO - 1),
                    )
                # rn = NS/||h||: rsqrt = NS*Exp(-0.5*Ln(s)) on the ACT
                # tables (DVE reciprocal is single-lane slow; the Ln/Exp
                # tables already bound the kernel's overall accuracy)
                l_c = small.tile([1, 512], F32, tag="l_c", name=f"l_c{t}{ch}", bufs=2)
                rn_c = small.tile([1, 512], BF, tag="rn_c", name=f"rn_c{t}{ch}", bufs=2)
                nc.scalar.activation(l_c[:], ssps[:], AF.Ln)
                nc.scalar.activation(rn_c[:], l_c[:], AF.Exp, scale=-0.5, bias=lnns[:])
                nc.scalar.dma_start(rn_dram[rn_slot : rn_slot + 1, sl], rn_c[:])
                rn_bc = scratch.tile([P, 512], BF, tag="rnbc", bufs=2, name=f"rn_bc{t}{ch}")
                nc.scalar.dma_start(
                    rn_bc[:],
                    rn_dram[rn_slot : rn_slot + 1, sl].to_broadcast((P, 512)),
                )
                for kt in range(KO):
                    nc.vector.tensor_tensor(
                        n_f8[:, kt, sl], n_sb[:, kt, sl], rn_bc[:], ALU.mult
                    )
                nc.scalar.dma_start(kp(ag_in[t, ch][:]), n_f8[:, :, sl])
                nc.gpsimd.collective_compute(
                    "AllGather", ALU.bypass, replica_groups=rg,
                    ins=[ag_in[t, ch][:].opt()], outs=[ag_out[t, ch][:].opt()],
                )

            elu1 = mats.tile([P, KO, BLK], F8, tag="elu")
            # own slots: n_f8 ch-0 writes must not WAR-wait on the elu/z slots'
            # ch-1 readers, or the early AllGather trigger serializes away
            n1_f8 = mats.tile([P, KO, BLK], F8, tag="n1f8", name="n1_f8")
            elu2 = mats.tile([P, KO, BLK], F8, tag="elu2", name="elu2")
            n2_f8 = mats.tile([P, KO, BLK], F8, tag="n2f8", name="n2_f8")
            z1_at = lambda kt, ch: z_sb[:, kt : kt + 2, bass.ds(ch * 512, 512)]
            z2_at = lambda kt, ch: (z2a if ch == 0 else z2b)[:, kt : kt + 2, :]
            # phase-sequential: each (tensor, half) runs L1 then L2+tail so its
            # AllGather triggers as early as possible (AG1a ~60us); the next
            # phase's L1 matmuls then cover the tail's activation drain
            for ch in (0, 1):
                proj_l1(z1_at, elu1, ch)
                proj_l2_tail(elu1, n1_sb, n1_f8, 0, 1, ch)
                proj_l1(z2_at, elu2, ch)
                proj_l2_tail(elu2, n2_sb, n2_f8, 1, 2, ch)


            # rowsum partials, one column per (half, jp-pair); S11 and S12
            # share one tile so a single reduce yields r11+r12
            r1x = strip.tile([P, NT, 2 * JP], F32)
            r22p = strip.tile([P, NT, JP], F32)
            cs = strip.tile([P, N], F32)  # exp(2*S12) partial column sums

            def rhs_one(t, h, jp, tag_n):
                r = rhsp.tile([P, KO, 512], F8, tag="rhs", name=f"rhs_{tag_n}")
                nc.sync.dma_start(r[:], kp(ag_out[t, h][jp]))
                return r

            def sim_iter(lhs, tt, rt0, rt1, accum, cs_sl=None):
                # fp8 DoubleRow: each matmul consumes a [P, 2, x] K-slab pair
                # (K=256); dots carry the NS^2 scale, undone in the exp scale.
                ps = psA.tile([P, 1024], F32, tag="ps_big", name="ps_sim")
                for kt in range(0, KO, 2):
                    for ch, rt in ((0, rt0), (1, rt1)):
                        nc.tensor.matmul(
                            ps[:, bass.ts(ch, 512)],
                            lhs[:, kt : kt + 2, bass.ts(tt, P)],
                            rt[:, kt : kt + 2, :],
                            start=(kt == 0),
                            stop=(kt == KO - 2),
                            perf_mode=DR,
                        )
                ex = expp.tile([P, 1024], F32, tag="ex")
                nc.scalar.activation(
                    ex[:], ps[:], AF.Exp, scale=2.0 / (NS * NS), accum_out=accum
                )
                if cs_sl is not None:
                    for ch in range(2):
                        nc.vector.tensor_tensor(
                            cs_sl[ch], cs_sl[ch], ex[:, bass.ts(ch, 512)], ALU.add
                        )

            def colsum_flush(jp, h):
                # cs chunk (jp, h) complete -> bf16 stage, reduce over partitions,
                # ship to the ReduceScatter input at its global-j offset.
                g = jp * 1024 + h * 512
                csb = scratch.tile([P, 512], BF, tag="csb", bufs=2, name=f"csb{jp}_{h}")
                nc.vector.tensor_copy(csb[:], cs[:, bass.ds(g, 512)])
                cp = psB.tile([1, 512], F32, tag="ps_small", name=f"cp{jp}_{h}")
                nc.tensor.matmul(cp[:], ones_bf[:], csb[:], start=True, stop=True)
                cst = scratch.tile([1, 512], F32, tag="cst", bufs=2, name=f"cst{jp}_{h}")
                nc.vector.tensor_copy(cst[:], cp[:])
                nc.gpsimd.dma_start(rs_in[g : g + 512], cst[:])

            def sim_pass(lhs, t, racc, is_s12, col_base=0):
                # phase h=0 runs entirely on the first AllGather half so it can
                # start before the second half lands; pair chunks share one exp.
                col = col_base
                for h in (0, 1):
                    for pj in range(0, JP, 2):
                        rt0 = rhs_one(t, h, pj, f"{t}{h}{pj}")
                        rt1 = rhs_one(t, h, pj + 1, f"{t}{h}{pj + 1}")
                        for tt in range(NT):
                            cs_sl = None
                            if is_s12:
                                cs_sl = (
                                    cs[:, bass.ds(pj * 1024 + h * 512, 512)],
                                    cs[:, bass.ds((pj + 1) * 1024 + h * 512, 512)],
                                )
                            sim_iter(lhs, tt, rt0, rt1, racc[:, tt, col : col + 1], cs_sl)
                        if is_s12:
                            colsum_flush(pj, h)
                            colsum_flush(pj + 1, h)
                        col += 1

            # ---------------- p_i = n1_i . n2_i (local diag of S12, x NS^2) ----------------
            pps = [psB.tile([1, 512], F32, name=f"pps{_c}", tag="ps_small") for _c in range(2)]
            for kt in range(KO):
                q = scratch.tile([P, BLK], BF, tag="sq")
                nc.vector.tensor_tensor(q[:], n1_f8[:, kt, :], n2_f8[:, kt, :], ALU.mult)
                for ch in range(2):
                    nc.tensor.matmul(
                        pps[ch][:],
                        ones_bf[:],
                        q[:, bass.ts(ch, 512)],
                        start=(kt == 0),
                        stop=(kt == KO - 1),
                    )
            for ch in range(2):
                p_c = small.tile([1, 512], F32, tag="p_c", name=f"p_c{ch}", bufs=2)
                nc.vector.tensor_copy(p_c[:], pps[ch][:])
                nc.gpsimd.dma_start(p_dram[ch * 512 : (ch + 1) * 512], p_c[:])
            # ---- pass B1: S12 (lhs n1, rhs gathered n2) + incremental colsums ----
            nc.vector.memset(cs[:], 0.0)
            sim_pass(n1_f8, 2, r1x, True, col_base=JP)
            nc.gpsimd.collective_compute(
                "ReduceScatter", ALU.add, replica_groups=rg,
                ins=[rs_in[:].opt()], outs=[rs_out[:].opt()],
            )
            # ---- pass A: S11 (lhs n1, rhs gathered n1) ----
            sim_pass(n1_f8, 1, r1x, False)


            # ---- pass B2: S22 (lhs n2, rhs gathered n2); RS overlaps this ----
            sim_pass(n2_f8, 2, r22p, False)

            # ---------------- final loss:  0.5*ln(d1*d2) - 2*p/NS^2 ----------------
            c12 = small.tile([P, NT], F32, tag="c12")
            nc.sync.dma_start(c12[:], pt(rs_out[:]))
            p2 = small.tile([P, NT], F32, tag="p2")
            nc.sync.dma_start(p2[:], pt(p_dram[:]))
            pm = small.tile([P, NT], F32, tag="pm")
            nc.vector.tensor_scalar(pm[:], p2[:], -2.0 / (NS * NS), None, ALU.mult)

            d1 = small.tile([P, NT], F32, tag="d1")
            d2 = small.tile([P, NT], F32, tag="d2")
            nc.vector.reduce_sum(d1[:], r1x[:], axis=mybir.AxisListType.X)
            nc.vector.tensor_scalar(d1[:], d1[:], -E2, None, ALU.add)
            nc.vector.reduce_sum(d2[:], r22p[:], axis=mybir.AxisListType.X)
            nc.vector.tensor_tensor(d2[:], d2[:], c12[:], ALU.add)
            nc.vector.tensor_scalar(d2[:], d2[:], -E2, None, ALU.add)
            nc.vector.tensor_tensor(d1[:], d1[:], d2[:], ALU.mult)
            lns = small.tile([P, NT], F32, tag="lns")
            nc.scalar.activation(lns[:], d1[:], AF.Ln)
            loss = small.tile([P, NT], F32, tag="loss")
            nc.vector.tensor_scalar(loss[:], lns[:], 0.5, None, ALU.mult)
            nc.vector.tensor_tensor(loss[:], loss[:], pm[:], ALU.add)
            nc.sync.dma_start(pt(out[:]), loss[:])

    nc.finalize()
    return nc


@lru_cache(maxsize=1)
def _built():
    return _build()


def _prep_inputs(z1, z2, fc1_w, fc1_b, fc2_w, fc2_b):
    f8 = ml_dtypes.float8_e4m3  # TRN FP8_EXP4-compatible below +-240
    w1t = np.ascontiguousarray(np.asarray(fc1_w, np.float32).T * WS).astype(f8)
    w2t = np.ascontiguousarray(np.asarray(fc2_w, np.float32).T * WS).astype(f8)
    b1 = np.asarray(fc1_b, np.float32)
    b2p = (np.asarray(fc2_b, np.float32) - np.asarray(fc2_w, np.float32).sum(axis=1)).astype(
        np.float32
    )
    in_maps = []
    for c in range(NCORES):
        sl = slice(c * BLK, (c + 1) * BLK)
        in_maps.append(
            {
                "z1t": np.ascontiguousarray(np.asarray(z1[sl], np.float32).T).astype(f8),
                "z2t": np.ascontiguousarray(np.asarray(z2[sl], np.float32).T).astype(f8),
                "w1t": w1t,
                "w2t": w2t,
                "b1": b1,
                "b2p": b2p,
            }
        )
    return in_maps


def _install_ntff_shim():
    """Register the axon NTFF profile hook (antenv.axon_hooks is absent in
    this image; rebuild it from trn_agent_boot's ctypes recipe)."""
    import sys
    import types

    if "antenv.axon_hooks" in sys.modules:
        return True
    try:
        import antenv
        from trn_agent_boot.trn_boot import _ntff_profile_via_ctypes

        hook = _ntff_profile_via_ctypes("/opt/axon/libaxon_pjrt.so")
        if hook is None:
            return False
        m = types.ModuleType("antenv.axon_hooks")
        m._hook = hook
        m.get_axon_ntff_profile_hook = lambda: m._hook
        m.set_axon_ntff_profile_hook = lambda h: setattr(m, "_hook", h)
        sys.modules["antenv.axon_hooks"] = m
        antenv.axon_hooks = m
        # artifact upload needs egress; neuter it for local profiling
        import concourse.bass_utils as _bu

        _bu.upload_artifacts = lambda tmpdir: f"file://{tmpdir}"
        return True
    except Exception as e:
        print(f"ntff shim unavailable: {e!r}")
        return False


def _run(in_maps, trace=False):
    nc = _built()
    if trace and not _install_ntff_shim():
        trace = False
    last = None
    for attempt in range(3):
        try:
            res = run_bass_kernel_spmd(nc, in_maps, list(range(NCORES)), trace=trace)
            if all(np.isfinite(res.results[c]["out"]).all() for c in range(NCORES)):
                return res
            print("nonfinite output, retrying")
        except Exception as e:  # device occasionally wedged from a prior process
            last = e
            if "UNRECOVERABLE" not in str(e) and "UNAVAILABLE" not in str(e):
                raise
            print(f"device error (attempt {attempt}): retrying")
    if last is not None:
        raise last
    return res


def kernel(z1, z2, fc1_w, fc1_b, fc2_w, fc2_b):
    in_maps = _prep_inputs(z1, z2, fc1_w, fc1_b, fc2_w, fc2_b)
    res = _run(in_maps, trace=os.environ.get("KERNEL_TRACE", "") == "1")
    if res.exec_time_ns is not None:
        print(f"HW exec time: {res.exec_time_ns} ns")
    out = np.concatenate([res.results[c]["out"] for c in range(NCORES)])
    return out.astype(np.float32)


# revision 50
# speedup vs baseline: 1.0397x; 1.0091x over previous
"""Trainium2 Bass kernel for nn_CLLayer (SimCLR-style contrastive loss).

Math (reference, tau=0.5):
    h1 = elu(z1 @ W1.T + b1) @ W2.T + b2 ; h2 likewise
    n1, n2 = row-normalized h1, h2
    l1_i = log(sum_j exp(2*n1_i.n1_j) + sum_j exp(2*n1_i.n2_j) - e^2) - 2*n1_i.n2_i
    l2_i = log(sum_j exp(2*n2_i.n2_j) + sum_j exp(2*n2_j.n1_i... ) - e^2) - 2*...
    out = 0.5*(l1+l2)

Sharding: row-parallel over N=8192 (1024 rows/core, 8 cores).
Each core: projects its row block, normalizes, AllGathers normalized
embeddings, computes its row-strip of the three distinct similarity
products (S12, S22, S11), exp+row-sums on the fly, column-sums of
exp(2*S12) via a ReduceScatter (between2 = between.T so l2's "between"
row sums are column sums of S12's exp).  Only 3 of 4 N^2*D products run.

All matmuls are fp8e4 with perf_mode=DoubleRow (2x PE throughput;
each MM consumes a [P, 2, x] K-slab pair, K=256).  fp8 subnormals are
avoided by x16 pre-scales: weights are scaled x16 on the host (undone
via the activation `scale`), normalized embeddings x16 on device
(undone in the exp scale 2/256 and the positive-pair term -2/256).
Each AllGather is split into two column halves so pass A can start on
the first half while the second is still in flight.

Host-side prep: transposes z blocks / weights to K-major (PE wants K on
partitions), casts matmul operands to fp8e4 (ml_dtypes.float8_e4m3
matches TRN FP8_EXP4 bit-exactly below 240), and folds the ELU "-1"
into an adjusted fc2 bias (b2' = b2 - fc2_w.sum(1)) so ELU is computed
as relu(x) + min(exp(x),1) without the subtract (device ELU' = elu+1).
"""

import math
import os
from functools import lru_cache

import ml_dtypes
import numpy as np

import concourse.bacc as bacc
import concourse.bass as bass
import concourse.mybir as mybir
import concourse.tile as tile
from concourse.bass_utils import run_bass_kernel_spmd

N, D = 8192, 1024
NCORES = 8
BLK = N // NCORES  # 1024
P = 128
KO = D // P  # 8 k-tiles
NT = BLK // P  # 8 i-tiles per core
JP = NCORES  # 8 j-chunks of 1024 (= core blocks)
E2 = float(np.exp(2.0))  # exp(1/tau), tau=0.5
BF = mybir.dt.bfloat16
F8 = mybir.dt.float8e4
F32 = mybir.dt.float32
NS = 16.0  # fp8 pre-scale on normalized embeddings
WS = 16.0  # fp8 pre-scale on weights (host side)
DR = mybir.MatmulPerfMode.DoubleRow
AF = mybir.ActivationFunctionType
ALU = mybir.AluOpType


def _build():
    nc = bacc.Bacc("TRN2", target_bir_lowering=False, debug=False, num_devices=NCORES)

    z1t = nc.dram_tensor("z1t", [D, BLK], F8, kind="ExternalInput")
    z2t = nc.dram_tensor("z2t", [D, BLK], F8, kind="ExternalInput")
    w1t = nc.dram_tensor("w1t", [D, D], F8, kind="ExternalInput")
    w2t = nc.dram_tensor("w2t", [D, D], F8, kind="ExternalInput")
    b1 = nc.dram_tensor("b1", [D], F32, kind="ExternalInput")
    b2p = nc.dram_tensor("b2p", [D], F32, kind="ExternalInput")
    out = nc.dram_tensor("out", [BLK], F32, kind="ExternalOutput")

    kp = lambda ap: ap.rearrange("(ko ki) x -> ki ko x", ki=P)  # K-major -> [128, KO, x]
    pt = lambda ap: ap.rearrange("(t p) -> p t", p=P)  # [1024] -> [128, 8]

    with tile.TileContext(nc) as tc:
        with (
            tc.tile_pool(name="consts", bufs=1) as consts,
            tc.tile_pool(name="mats", bufs=1) as mats,
            tc.tile_pool(name="strip", bufs=1) as strip,
            tc.tile_pool(name="scratch", bufs=2) as scratch,
            tc.tile_pool(name="rhs", bufs=6) as rhsp,
            tc.tile_pool(name="expp", bufs=2) as expp,
            tc.tile_pool(name="small", bufs=1) as small,
            tc.tile_pool(name="psA", bufs=3, space="PSUM") as psA,
            tc.tile_pool(name="psB", bufs=2, space="PSUM") as psB,
            tc.tile_pool(name="dram", bufs=1, space="DRAM") as dram,
        ):
            # ---------------- constants / inputs (proj1's needs first) ----------------
            w1_sb = consts.tile([P, KO, D], F8)
            w2_sb = consts.tile([P, KO, D], F8)
            b1_sb = consts.tile([P, KO], F32)
            b2_sb = consts.tile([P, KO], F32)
            z_sb = mats.tile([P, KO, BLK], F8, tag="zt")
            # halved transfers so proj1's first matmuls start sooner
            nc.sync.dma_start(w1_sb[:, :, 0:512], kp(w1t[:])[:, :, 0:512])
            nc.sync.dma_start(z_sb[:, :, 0:512], kp(z1t[:])[:, :, 0:512])
            nc.sync.dma_start(w1_sb[:, :, 512:1024], kp(w1t[:])[:, :, 512:1024])
            nc.sync.dma_start(z_sb[:, :, 512:1024], kp(z1t[:])[:, :, 512:1024])
            nc.sync.dma_start(b1_sb[:], pt(b1[:]))
            nc.sync.dma_start(w2_sb[:], kp(w2t[:]))
            nc.sync.dma_start(b2_sb[:], pt(b2p[:]))
            # own tag: staging must not pin rhs-pool slots (WAR on the pool
            # rotation would stall pass A's prefetch until proj2-ch1 retires)
            z2a = rhsp.tile([P, KO, 512], F8, tag="zstage", name="z2a", bufs=2)
            z2b = rhsp.tile([P, KO, 512], F8, tag="zstage", name="z2b", bufs=2)
            nc.sync.dma_start(z2a[:], kp(z2t[:, 0:512]))
            nc.sync.dma_start(z2b[:], kp(z2t[:, 512:1024]))
            ones_bf = consts.tile([P, 1], BF)
            nc.vector.memset(ones_bf[:], 1.0)
            lnns = consts.tile([1, 1], F32)
            nc.vector.memset(lnns[:], float(math.log(NS)))

            n1_sb = mats.tile([P, KO, BLK], BF, tag="n1")
            n2_sb = mats.tile([P, KO, BLK], BF, tag="n2")

            ag_in = {}
            ag_out = {}
            for t in (1, 2):
                for h in (0, 1):
                    ag_in[t, h] = dram.tile([D, 512], F8, name=f"ag{t}{h}_in")
                    ag_out[t, h] = dram.tile(
                        [NCORES, D, 512], F8, addr_space="Shared", name=f"ag{t}{h}_out"
                    )
            rs_in = dram.tile([N], F32)
            rs_out = dram.tile([BLK], F32)
            rn_dram = dram.tile([2, BLK], BF)
            p_dram = dram.tile([BLK], F32)

            rg = [list(range(NCORES))]

            # ------------ projection + normalize (into n_sb + n_f8), per tensor ------------
            # Column-half-outer: each 512-column half runs L1 -> L2 -> sumsq ->
            # rn -> fp8 cast -> its AllGather trigger before the other half
            # starts, so AG h=0 is in flight ~half a projection early and
            # collective-duration variance hides under the remaining compute.
            def proj_l1(z_at, elu_sb, ch):
                # layer 1: a1T[o, i] = W1T.T @ zT (K=d);
                # elu+1 = relu(y) + min(exp(y), 1), y = ps/WS + b1
                sl = bass.ds(ch * 512, 512)
                for ot in range(KO):
                    ps = psA.tile([P, 512], F32, tag="ps_big", name="ps_l1")
                    for kt in range(0, KO, 2):
                        nc.tensor.matmul(
                            ps[:],
                            w1_sb[:, kt : kt + 2, bass.ts(ot, P)],
                            z_at(kt, ch),
                            start=(kt == 0),
                            stop=(kt == KO - 2),
                            perf_mode=DR,
                        )
                    bcol = b1_sb[:, ot : ot + 1]
                    e_t = scratch.tile([P, 512], F32, tag="e_t")
                    r_t = scratch.tile([P, 512], F32, tag="r_t")
                    nc.scalar.activation(e_t[:], ps[:], AF.Exp, bias=bcol, scale=1.0 / WS)
                    nc.scalar.activation(r_t[:], ps[:], AF.Relu, bias=bcol, scale=1.0 / WS)
                    nc.vector.tensor_scalar(e_t[:], e_t[:], 1.0, None, ALU.min)
                    nc.vector.tensor_tensor(elu_sb[:, ot, sl], e_t[:], r_t[:], ALU.add)

            def proj_l2_tail(elu_sb, n_sb, n_f8, rn_slot, t, ch):
                sl = bass.ds(ch * 512, 512)
                # layer 2 -> n_sb (holds hT until scaled)
                ssps = psB.tile([1, 512], F32, name=f"ssps{t}{ch}", tag="ps_small")
                for ot in range(KO):
                    ps = psA.tile([P, 512], F32, tag="ps_big", name="ps_l2")
                    for kt in range(0, KO, 2):
                        nc.tensor.matmul(
                            ps[:],
                            w2_sb[:, kt : kt + 2, bass.ts(ot, P)],
                            elu_sb[:, kt : kt + 2, sl],
                            start=(kt == 0),
                            stop=(kt == KO - 2),
                            perf_mode=DR,
                        )
                    nc.vector.tensor_scalar(
                        n_sb[:, ot, sl], ps[:], 1.0 / WS, b2_sb[:, ot : ot + 1],
                        ALU.mult, ALU.add,
                    )
                    # sumsq over d (partitions) via ones-matmul on Square(h)
                    sq = scratch.tile([P, 512], BF, tag="sq")
                    nc.scalar.activation(sq[:], n_sb[:, ot, sl], AF.Square)
                    nc.tensor.matmul(
                        ssps[:], ones_bf[:], sq[:],
                        start=(ot == 0), stop=(ot == KO - 1),
                    )
                # rn = NS/||h||: rsqrt = NS*Exp(-0.5*Ln(s)) on the ACT
                # tables (DVE reciprocal is single-lane slow; the Ln/Exp
                # tables already bound the kernel's overall accuracy)
                l_c = small.tile([1, 512], F32, tag="l_c", name=f"l_c{t}{ch}", bufs=2)
                rn_c = small.tile([1, 512], BF, tag="rn_c", name=f"rn_c{t}{ch}", bufs=2)
                nc.scalar.activation(l_c[:], ssps[:], AF.Ln)
                nc.scalar.activation(rn_c[:], l_c[:], AF.Exp, scale=-0.5, bias=lnns[:])
                nc.scalar.dma_start(rn_dram[rn_slot : rn_slot + 1, sl], rn_c[:])
                rn_bc = scratch.tile([P, 512], BF, tag="rnbc", bufs=2, name=f"rn_bc{t}{ch}")
                nc.scalar.dma_start(
                    rn_bc[:],
                    rn_dram[rn_slot : rn_slot + 1, sl].to_broadcast((P, 512)),
                )
                for kt in range(KO):
                    nc.vector.tensor_tensor(
                        n_f8[:, kt, sl], n_sb[:, kt, sl], rn_bc[:], ALU.mult
                    )
                nc.scalar.dma_start(kp(ag_in[t, ch][:]), n_f8[:, :, sl])
                nc.gpsimd.collective_compute(
                    "AllGather", ALU.bypass, replica_groups=rg,
                    ins=[ag_in[t, ch][:].opt()], outs=[ag_out[t, ch][:].opt()],
                )

            elu1 = mats.tile([P, KO, BLK], F8, tag="elu")
            # own slots: n_f8 ch-0 writes must not WAR-wait on the elu/z slots'
            # ch-1 readers, or the early AllGather trigger serializes away
            n1_f8 = mats.tile([P, KO, BLK], F8, tag="n1f8", name="n1_f8")
            elu2 = mats.tile([P, KO, BLK], F8, tag="elu2", name="elu2")
            n2_f8 = mats.tile([P, KO, BLK], F8, tag="n2f8", name="n2_f8")
            z1_at = lambda kt, ch: z_sb[:, kt : kt + 2, bass.ds(ch * 512, 512)]
            z2_at = lambda kt, ch: (z2a if ch == 0 else z2b)[:, kt : kt + 2, :]
            # phase-sequential: each (tensor, half) runs L1 then L2+tail so its
            # AllGather triggers as early as possible (AG1a ~60us); the next
            # phase's L1 matmuls then cover the tail's activation drain
            for ch in (0, 1):
                proj_l1(z1_at, elu1, ch)
                proj_l2_tail(elu1, n1_sb, n1_f8, 0, 1, ch)
                proj_l1(z2_at, elu2, ch)
                proj_l2_tail(elu2, n2_sb, n2_f8, 1, 2, ch)


            # rowsum partials, one column per (half, jp-pair); S11 and S12
            # share one tile so a single reduce yields r11+r12
            r1x = strip.tile([P, NT, 2 * JP], F32)
            r22p = strip.tile([P, NT, JP], F32)
            cs = strip.tile([P, N], F32)  # exp(2*S12) partial column sums

            def rhs_one(t, h, jp, tag_n):
                r = rhsp.tile([P, KO, 512], F8, tag="rhs", name=f"rhs_{tag_n}")
                nc.sync.dma_start(r[:], kp(ag_out[t, h][jp]))
                return r

            def sim_iter(lhs, tt, rt0, rt1, accum, cs_sl=None):
                # fp8 DoubleRow: each matmul consumes a [P, 2, x] K-slab pair
                # (K=256); dots carry the NS^2 scale, undone in the exp scale.
                ps = psA.tile([P, 1024], F32, tag="ps_big", name="ps_sim")
                for kt in range(0, KO, 2):
                    for ch, rt in ((0, rt0), (1, rt1)):
                        nc.tensor.matmul(
                            ps[:, bass.ts(ch, 512)],
                            lhs[:, kt : kt + 2, bass.ts(tt, P)],
                            rt[:, kt : kt + 2, :],
                            start=(kt == 0),
                            stop=(kt == KO - 2),
                            perf_mode=DR,
                        )
                ex = expp.tile([P, 1024], F32, tag="ex")
                nc.scalar.activation(
                    ex[:], ps[:], AF.Exp, scale=2.0 / (NS * NS), accum_out=accum
                )
                if cs_sl is not None:
                    for ch in range(2):
                        nc.vector.tensor_tensor(
                            cs_sl[ch], cs_sl[ch], ex[:, bass.ts(ch, 512)], ALU.add
                        )

            def colsum_flush(jp, h):
                # cs chunk (jp, h) complete -> bf16 stage, reduce over partitions,
                # ship to the ReduceScatter input at its global-j offset.
                g = jp * 1024 + h * 512
                csb = scratch.tile([P, 512], BF, tag="csb", bufs=2, name=f"csb{jp}_{h}")
                nc.vector.tensor_copy(csb[:], cs[:, bass.ds(g, 512)])
                cp = psB.tile([1, 512], F32, tag="ps_small", name=f"cp{jp}_{h}")
                nc.tensor.matmul(cp[:], ones_bf[:], csb[:], start=True, stop=True)
                cst = scratch.tile([1, 512], F32, tag="cst", bufs=2, name=f"cst{jp}_{h}")
                nc.vector.tensor_copy(cst[:], cp[:])
                nc.gpsimd.dma_start(rs_in[g : g + 512], cst[:])

            def sim_pass(lhs, t, racc, is_s12, col_base=0):
                # phase h=0 runs entirely on the first AllGather half so it can
                # start before the second half lands; pair chunks share one exp.
                col = col_base
                for h in (0, 1):
                    for pj in range(0, JP, 2):
                        rt0 = rhs_one(t, h, pj, f"{t}{h}{pj}")
                        rt1 = rhs_one(t, h, pj + 1, f"{t}{h}{pj + 1}")
                        for tt in range(NT):
                            cs_sl = None
                            if is_s12:
                                cs_sl = (
                                    cs[:, bass.ds(pj * 1024 + h * 512, 512)],
                                    cs[:, bass.ds((pj + 1) * 1024 + h * 512, 512)],
                                )
                            sim_iter(lhs, tt, rt0, rt1, racc[:, tt, col : col + 1], cs_sl)
                        if is_s12:
                            colsum_flush(pj, h)
                            colsum_flush(pj + 1, h)
                        col += 1

            # ---------------- p_i = n1_i . n2_i (local diag of S12, x NS^2) ----------------
            pps = [psB.tile([1, 512], F32, name=f"pps{_c}", tag="ps_small") for _c in range(2)]
            for kt in range(KO):
                q = scratch.tile([P, BLK], BF, tag="sq")
                nc.vector.tensor_tensor(q[:], n1_f8[:, kt, :], n2_f8[:, kt, :], ALU.mult)
                for ch in range(2):
                    nc.tensor.matmul(
                        pps[ch][:],
                        ones_bf[:],
                        q[:, bass.ts(ch, 512)],
                        start=(kt == 0),
                        stop=(kt == KO - 1),
                    )
            for ch in range(2):
                p_c = small.tile([1, 512], F32, tag="p_c", name=f"p_c{ch}", bufs=2)
                nc.vector.tensor_copy(p_c[:], pps[ch][:])
                nc.gpsimd.dma_start(p_dram[ch * 512 : (ch + 1) * 512], p_c[:])
            # ---- pass B1: S12 (lhs n1, rhs gathered n2) + incremental colsums ----
            nc.vector.memset(cs[:], 0.0)
            sim_pass(n1_f8, 2, r1x, True, col_base=JP)
            nc.gpsimd.collective_compute(
                "ReduceScatter", ALU.add, replica_groups=rg,
                ins=[rs_in[:].opt()], outs=[rs_out[:].opt()],
            )
            # ---- pass A: S11 (lhs n1, rhs gathered n1) ----
            sim_pass(n1_f8, 1, r1x, False)


            # ---- pass B2: S22 (lhs n2, rhs gathered n2); RS overlaps this ----
            sim_pass(n2_f8, 2, r22p, False)

            # ---------------- final loss:  0.5*ln(d1*d2) - 2*p/NS^2 ----------------
            c12 = small.tile([P, NT], F32, tag="c12")
            nc.sync.dma_start(c12[:], pt(rs_out[:]))
            p2 = small.tile([P, NT], F32, tag="p2")
            nc.sync.dma_start(p2[:], pt(p_dram[:]))
            pm = small.tile([P, NT], F32, tag="pm")
            nc.vector.tensor_scalar(pm[:], p2[:], -2.0 / (NS * NS), None, ALU.mult)

            d1 = small.tile([P, NT], F32, tag="d1")
            d2 = small.tile([P, NT], F32, tag="d2")
            nc.vector.reduce_sum(d1[:], r1x[:], axis=mybir.AxisListType.X)
            nc.vector.tensor_scalar(d1[:], d1[:], -E2, None, ALU.add)
            nc.vector.reduce_sum(d2[:], r22p[:], axis=mybir.AxisListType.X)
            nc.vector.tensor_tensor(d2[:], d2[:], c12[:], ALU.add)
            nc.vector.tensor_scalar(d2[:], d2[:], -E2, None, ALU.add)
            nc.vector.tensor_tensor(d1[:], d1[:], d2[:], ALU.mult)
            lns = small.tile([P, NT], F32, tag="lns")
            nc.scalar.activation(lns[:], d1[:], AF.Ln)
            loss = small.tile([P, NT], F32, tag="loss")
            nc.vector.tensor_scalar(loss[:], lns[:], 0.5, None, ALU.mult)
            nc.vector.tensor_tensor(loss[:], loss[:], pm[:], ALU.add)
            nc.sync.dma_start(pt(out[:]), loss[:])

    nc.finalize()
    return nc


@lru_cache(maxsize=1)
def _built():
    return _build()


def _prep_inputs(z1, z2, fc1_w, fc1_b, fc2_w, fc2_b):
    f8 = ml_dtypes.float8_e4m3  # TRN FP8_EXP4-compatible below +-240
    w1t = np.ascontiguousarray(np.asarray(fc1_w, np.float32).T * WS).astype(f8)
    w2t = np.ascontiguousarray(np.asarray(fc2_w, np.float32).T * WS).astype(f8)
    b1 = np.asarray(fc1_b, np.float32)
    b2p = (np.asarray(fc2_b, np.float32) - np.asarray(fc2_w, np.float32).sum(axis=1)).astype(
        np.float32
    )
    in_maps = []
    for c in range(NCORES):
        sl = slice(c * BLK, (c + 1) * BLK)
        in_maps.append(
            {
                "z1t": np.ascontiguousarray(np.asarray(z1[sl], np.float32).T).astype(f8),
                "z2t": np.ascontiguousarray(np.asarray(z2[sl], np.float32).T).astype(f8),
                "w1t": w1t,
                "w2t": w2t,
                "b1": b1,
                "b2p": b2p,
            }
        )
    return in_maps


def _install_ntff_shim():
    """Register the axon NTFF profile hook (antenv.axon_hooks is absent in
    this image; rebuild it from trn_agent_boot's ctypes recipe)."""
    import sys
    import types

    if "antenv.axon_hooks" in sys.modules:
        return True
    try:
        import antenv
        from trn_agent_boot.trn_boot import _ntff_profile_via_ctypes

        hook = _ntff_profile_via_ctypes("/opt/axon/libaxon_pjrt.so")
        if hook is None:
            return False
        m = types.ModuleType("antenv.axon_hooks")
        m._hook = hook
        m.get_axon_ntff_profile_hook = lambda: m._hook
        m.set_axon_ntff_profile_hook = lambda h: setattr(m, "_hook", h)
        sys.modules["antenv.axon_hooks"] = m
        antenv.axon_hooks = m
        # artifact upload needs egress; neuter it for local profiling
        import concourse.bass_utils as _bu

        _bu.upload_artifacts = lambda tmpdir: f"file://{tmpdir}"
        return True
    except Exception as e:
        print(f"ntff shim unavailable: {e!r}")
        return False


def _run(in_maps, trace=False):
    nc = _built()
    if trace and not _install_ntff_shim():
        trace = False
    last = None
    for attempt in range(3):
        try:
            res = run_bass_kernel_spmd(nc, in_maps, list(range(NCORES)), trace=trace)
            if all(np.isfinite(res.results[c]["out"]).all() for c in range(NCORES)):
                return res
            print("nonfinite output, retrying")
        except Exception as e:  # device occasionally wedged from a prior process
            last = e
            if "UNRECOVERABLE" not in str(e) and "UNAVAILABLE" not in str(e):
                raise
            print(f"device error (attempt {attempt}): retrying")
    if last is not None:
        raise last
    return res


def kernel(z1, z2, fc1_w, fc1_b, fc2_w, fc2_b):
    in_maps = _prep_inputs(z1, z2, fc1_w, fc1_b, fc2_w, fc2_b)
    res = _run(in_maps, trace=os.environ.get("KERNEL_TRACE", "") == "1")
    if res.exec_time_ns is not None:
        print(f"HW exec time: {res.exec_time_ns} ns")
    out = np.concatenate([res.results[c]["out"] for c in range(NCORES)])
    return out.astype(np.float32)
